# revision 34
# baseline (speedup 1.0000x reference)
"""CfC (closed-form continuous-time) cell kernel for Trainium2, 8 NeuronCores.

Reference computation (B=8192, IN=256, H=512, all fp32):
    g     = sigmoid(x @ W_gx.T + b_gx + h @ W_gh.T + gate_b)        [B, H]
    f     = tanh(cat([x, h]) @ W_backbone.T + b_backbone)           [B, H]
    tau   = softplus(log_tau) + |g|          (g in (0,1) so |g| == g)
    decay = exp(-delta_t[:, None] * tau)
    out   = decay * h + (1 - decay) * f

Strategy: data-parallel over B (1024 rows per core), weights replicated.
Feature-major on device: activations ship as xh^T [768, B_shard] so the
contraction dim lands on SBUF partitions with no on-device transposes.

Matmul precision split: the GATE matmul runs in fp8-e4m3 DoubleRow perf
mode (2x PE rate; the gate feeds sigmoid -> tau -> exp, so its error is
strongly attenuated), while the BACKBONE matmul (direct path to the output)
stays fp16.  Gate operands are pre-scaled by powers of two (x*8, W*32) to
center them in e4m3's normal range; the 1/256 dequant plus the sigmoid
half-angle 1/2 fold into the ACT scale (1/512).

Two-phase schedule, sized by the input-DMA roofline (~3.7MB at ~300GB/s):
the gate phase needs only the small fp8 streams (1.15MB) so it starts as
soon as they land; its epilogue chain (tg -> t -> decay) has no backbone
dependency.  The backbone phase (fp16, 2.3MB) streams in behind it.  Both
phases emit matmuls j-tile by j-tile through a single 4-deep PSUM ring.

    phase A, per j:  zg = DoubleRow-fp8 matmuls
                     tg    = Tanh(zg/512 + bg/2)              ACT
                     t     = (tg + 2*softplus'+1) * (-dt/2)   DVE STT
                     decay = Exp(t)                           ACT
    phase B, per j:  zf = fp16 matmuls
                     f     = Tanh(zf + bb)                    ACT
                     hmf   = h - f                            DVE
                     p     = decay * hmf                      DVE
                     o     = p + f -> DMA out (fp16)          DVE

softplus(log_tau) is a [H] constant computed on the host, so the scalar
engine needs one ACT table set (tanh+exp) loaded once.  All 16-bit DVE ops
hit the 2x packed mode.  Activations are packed chunk-major so each DMA is
one contiguous-row panel in matmul consumption order; triggers are split
across the two HWDGE rings (sync: gate stream, scalar: backbone stream)
plus gpsimd SWDGE for -dt/2.  The last j-tile runs a chunk-split epilogue
to shorten the post-matmul tail.  Output ships fp16, upcast on the host.
"""

from contextlib import ExitStack

import ml_dtypes
import numpy as np

import concourse.bass as bass
import concourse.mybir as mybir
import concourse.tile as tile
from concourse import bacc
from concourse.bass_utils import run_bass_kernel_spmd

B, IN, H = 8192, 256, 512
NCORES = 8
BS = B // NCORES          # 1024 batch rows per core
KIN = IN + H              # 768 contraction dim
KT = KIN // 128           # 6 k-tiles
NP = KT // 2              # 3 k-pair panels (DoubleRow processes 2 k-tiles)
NJ = H // 128             # 4 partition tiles per output matrix
NCHUNK = 512              # matmul moving free dim per PSUM bank
NCH = BS // NCHUNK        # 2 b-chunks per core

SX = 8.0                  # gate activation pre-scale (power of 2)
SW = 32.0                 # gate weight pre-scale (power of 2)
GDEQ = 1.0 / (SX * SW * 2.0)   # ACT scale: dequant + sigmoid half-angle

F32 = mybir.dt.float32
FP16 = mybir.dt.float16
FP8 = mybir.dt.float8e4
AF = mybir.ActivationFunctionType
OP = mybir.AluOpType
PM = mybir.MatmulPerfMode

TRACE = False             # test.py flips this for profiled runs
LAST_RESULT = None        # BassKernelResults of the most recent run

_NC_CACHE = None


def _body(tc, gc8, xgc1, bc16, xhc1, ndt2, consts, outP):
    nc = tc.nc
    with ExitStack() as ctx:
        singles = ctx.enter_context(tc.tile_pool(name="singles", bufs=1))
        decs = ctx.enter_context(tc.tile_pool(name="decs", bufs=2))
        work = ctx.enter_context(tc.tile_pool(name="work", bufs=2))
        psg = ctx.enter_context(tc.tile_pool(name="psg", bufs=2, space="PSUM"))
        psf = ctx.enter_context(tc.tile_pool(name="psf", bufs=2, space="PSUM"))

        # Persistent SBUF tensors.  Activation streams are chunk-panel
        # tiles so each DMA is one contiguous-row panel in matmul
        # consumption order.
        gc_sb = singles.tile([128, 2, KT, NCHUNK], FP8, tag="gc")
        wg_sb = gc_sb[:, 0]
        xg0_sb = gc_sb[:, 1]
        xg1_sb = singles.tile([128, KT, NCHUNK], FP8, tag="xg1")
        wb_sb = singles.tile([128, KT, H], FP16, tag="wb")
        xh0_sb = singles.tile([128, KT, NCHUNK], FP16, tag="xh0")
        xh1_sb = singles.tile([128, KT, NCHUNK], FP16, tag="xh1")
        ndt_sb = singles.tile([128, BS], FP16, tag="ndt")
        cst = singles.tile([128, 3, NJ], F32, tag="cst")

        wg_v = wg_sb
        xg_v = {0: xg0_sb, 1: xg1_sb}           # per-chunk moving panels
        wb_v = wb_sb
        xh_v = {0: xh0_sb, 1: xh1_sb}

        CC = KT * NCHUNK
        # All input DMAs ride ONE HWDGE ring (sync) in exact consumption
        # order: the SDMA engines round-robin between rings at packet
        # granularity, so two concurrent rings starve whichever stream is
        # needed first.  One ring drains FIFO at full fabric rate.
        nc.sync.dma_start(out=gc_sb.rearrange("p s k c -> p (s k c)"),
                          in_=gc8)
        nc.sync.dma_start(out=xg1_sb.rearrange("p k c -> p (k c)"),
                          in_=xgc1)
        nc.sync.dma_start(out=wb_sb.rearrange("p k n -> p (k n)"),
                          in_=bc16[:, 0:CC])
        nc.sync.dma_start(out=xh0_sb.rearrange("p k c -> p (k c)"),
                          in_=bc16[:, CC:2 * CC])
        nc.sync.dma_start(out=xh1_sb.rearrange("p k c -> p (k c)"),
                          in_=xhc1)
        # Epilogue constants + -dt/2 broadcast ride the gpsimd SWDGE ring
        # (48B rows would clog the head of the main input ring).
        nc.gpsimd.dma_start(out=cst.rearrange("p c j -> p (c j)"),
                            in_=consts)
        nc.gpsimd.dma_start(out=ndt_sb, in_=ndt2)

        # --- Phase A: gate matmuls + gate chain (tg -> t -> decay) ---
        # t for j-pairs lands in one [128, 2*BS] tile so Exp runs as two
        # double-width ACTs (halves the per-instruction ACT overhead).
        # The otherwise-idle DVE then precomputes em = 1-decay and
        # dech = decay*h, so each phase-B combine is only two hops after
        # its tanh: o = f*em + dech.
        em = {}
        dech = {}
        tp = {}
        for j in range(NJ):
            zg = psg.tile([128, BS], F32, tag="zg", name=f"zg_{j}")
            for n in range(NCH):
                bsl = slice(n * NCHUNK, (n + 1) * NCHUNK)
                for g in range(NP):
                    nc.tensor.matmul(
                        zg[:, bsl],
                        wg_v[:, 2 * g:2 * g + 2, j * 128:(j + 1) * 128],
                        xg_v[n][:, 2 * g:2 * g + 2, :],
                        start=(g == 0),
                        stop=(g == NP - 1),
                        perf_mode=PM.DoubleRow,
                    )
            tg = work.tile([128, BS], FP16, tag="tg", name=f"tg_{j}")
            if j % 2 == 0:
                tp[j // 2] = work.tile([128, 2 * BS], FP16, tag="t",
                                       name=f"t_{j // 2}")
            # tg = tanh((zg + 256*bg)/512) = tanh(zg_true/2 + bg/2)
            nc.scalar.activation(
                out=tg, in_=zg, func=AF.Tanh, bias=cst[:, 0, j:j + 1],
                scale=GDEQ,
            )
            # t = (tg + (2*softplus+1)) * (-dt/2)  [= -dt * (softplus + g)]
            nc.vector.scalar_tensor_tensor(
                out=tp[j // 2][:, (j % 2) * BS:(j % 2 + 1) * BS],
                in0=tg, scalar=cst[:, 2, j:j + 1], in1=ndt_sb,
                op0=OP.add, op1=OP.mult,
            )
            if j % 2 == 1:
                dp = decs.tile([128, 2 * BS], FP16, tag="dec",
                               name=f"dec_{j // 2}")
                nc.scalar.activation(out=dp, in_=tp[j // 2], func=AF.Exp)
                ep = decs.tile([128, 2 * BS], FP16, tag="em",
                               name=f"em_{j // 2}")
                nc.vector.tensor_scalar(
                    out=ep, in0=dp, scalar1=-1.0, scalar2=1.0,
                    op0=OP.mult, op1=OP.add,
                )
                for jj in (j - 1, j):
                    em[jj] = ep[:, (jj % 2) * BS:(jj % 2 + 1) * BS]
                    dh = decs.tile([128, BS], FP16, tag="dech",
                                   name=f"dech_{jj}")
                    for c in range(NCH):
                        nc.vector.tensor_mul(
                            out=dh[:, c * NCHUNK:(c + 1) * NCHUNK],
                            in0=dp[:, (jj % 2) * BS + c * NCHUNK:
                                   (jj % 2) * BS + (c + 1) * NCHUNK],
                            in1=xh_v[c][:, 2 + jj, :],
                        )
                    dech[jj] = dh

        # --- Phase B: backbone matmuls + f + combine ---
        # Full-width combines for j<3; the last j-tile splits its epilogue
        # into one 512 chunk and two 256 half-chunks so every hop on the
        # final critical path gets cheaper.
        def combine(j, zf, csl, name):
            w = csl.stop - csl.start
            f = work.tile([128, w], FP16, tag=f"f{w}", name=f"f_{name}")
            p = work.tile([128, w], FP16, tag=f"p{w}", name=f"p_{name}")
            o = work.tile([128, w], FP16, tag=f"o{w}", name=f"o_{name}")
            nc.scalar.activation(
                out=f, in_=zf[:, csl], func=AF.Tanh, bias=cst[:, 1, j:j + 1]
            )
            # o = f*(1-decay) + decay*h, both factors precomputed in phase A
            nc.vector.tensor_mul(out=p, in0=f, in1=em[j][:, csl])
            nc.vector.tensor_add(out=o, in0=p, in1=dech[j][:, csl])
            nc.sync.dma_start(
                out=outP[:, j * BS + csl.start:j * BS + csl.stop], in_=o
            )

        HC = NCHUNK // 2
        for j in range(NJ):
            zf = psf.tile([128, BS], F32, tag="zf", name=f"zf_{j}")
            for n in range(NCH):
                bsl = slice(n * NCHUNK, (n + 1) * NCHUNK)
                for k in range(KT):
                    nc.tensor.matmul(
                        zf[:, bsl],
                        wb_v[:, k, j * 128:(j + 1) * 128],
                        xh_v[n][:, k, :],
                        start=(k == 0),
                        stop=(k == KT - 1),
                    )
            if j < NJ - 1:
                combine(j, zf, slice(0, BS), f"{j}")
            else:
                # Last tile: one full-width tanh, then quarter-width V
                # chains + DMAs so the final hops are as cheap as possible.
                f = work.tile([128, BS], FP16, tag="f1024", name="f_3")
                nc.scalar.activation(
                    out=f, in_=zf, func=AF.Tanh, bias=cst[:, 1, j:j + 1]
                )
                for q in range(2):
                    csl = slice(q * (BS // 2), (q + 1) * (BS // 2))
                    w = BS // 2
                    p = work.tile([128, w], FP16, tag="pq", name=f"p_3_{q}")
                    o = work.tile([128, w], FP16, tag="oq", name=f"o_3_{q}")
                    nc.vector.tensor_mul(out=p, in0=f[:, csl],
                                         in1=em[j][:, csl])
                    nc.vector.tensor_add(out=o, in0=p, in1=dech[j][:, csl])
                    nc.sync.dma_start(
                        out=outP[:, j * BS + csl.start:j * BS + csl.stop],
                        in_=o
                    )


def build_nc():
    nc = bacc.Bacc(
        "TRN2",
        target_bir_lowering=False,
        debug=False,
        enable_asserts=False,
        num_devices=NCORES,
    )
    # Partition-major packed streams: row p holds that partition's entire
    # contiguous payload.
    CC = KT * NCHUNK
    gc8 = nc.dram_tensor("gc8", [128, 2 * CC], FP8, kind="ExternalInput").ap()
    xgc1 = nc.dram_tensor("xgc1", [128, CC], FP8, kind="ExternalInput").ap()
    bc16 = nc.dram_tensor("bc16", [128, 2 * CC], FP16, kind="ExternalInput").ap()
    xhc1 = nc.dram_tensor("xhc1", [128, CC], FP16, kind="ExternalInput").ap()
    ndt2 = nc.dram_tensor("ndt2", [128, BS], FP16, kind="ExternalInput").ap()
    consts = nc.dram_tensor("consts", [128, 3 * NJ], F32,
                            kind="ExternalInput").ap()
    outP = nc.dram_tensor("outP", [128, NJ * BS], FP16, kind="ExternalOutput").ap()
    with tile.TileContext(nc) as tc:
        _body(tc, gc8, xgc1, bc16, xhc1, ndt2, consts, outP)
    nc.compile()
    return nc


def _get_nc():
    global _NC_CACHE
    if _NC_CACHE is None:
        _NC_CACHE = build_nc()
    return _NC_CACHE


def _pack_cmajor(a, kt, nch, nchunk):
    """[kt*128, nch*nchunk] -> [128, nch*kt*nchunk] chunk-major pack: row p
    holds [chunk0: k0..k5 | chunk1: k0..k5], each 128-partition-sliced."""
    return np.ascontiguousarray(
        a.reshape(kt, 128, nch, nchunk).transpose(1, 2, 0, 3)
        .reshape(128, nch * kt * nchunk)
    )


def _pack_pmajor(a, kt):
    """[kt*128, C] -> [128, kt*C]: partition-major pack so each of the 128
    DMA rows is contiguous in DRAM."""
    c = a.shape[1]
    return np.ascontiguousarray(
        a.reshape(kt, 128, c).transpose(1, 0, 2).reshape(128, kt * c)
    )


def make_in_maps(x, h, delta_t, W_backbone, b_backbone, W_gx, b_gx, W_gh,
                 gate_b, log_tau):
    f32 = np.float32
    xh = np.concatenate(
        [np.asarray(x, f32), np.asarray(h, f32)], axis=1
    )                                                   # [B, 768]
    xhT = np.ascontiguousarray(xh.T)                    # [768, B] f32
    xh16 = xhT.astype(np.float16)
    xg8 = np.asarray(xhT * SX, dtype=ml_dtypes.float8_e4m3)

    WgT = np.concatenate(
        [np.asarray(W_gx, f32), np.asarray(W_gh, f32)], axis=1
    ).T                                                 # [768, H]
    w8g = _pack_pmajor(np.asarray(WgT * SW, dtype=ml_dtypes.float8_e4m3), KT)
    w16b = _pack_pmajor(
        np.ascontiguousarray(np.asarray(W_backbone, f32).T).astype(np.float16),
        KT,
    )

    sp2 = 2.0 * np.log1p(np.exp(np.asarray(log_tau, f32))) + 1.0
    # cstP[p, c*NJ+j] = const_c[j*128+p]
    cstv = np.stack(
        [
            (np.asarray(b_gx, f32) + np.asarray(gate_b, f32)) * 0.5,
            np.asarray(b_backbone, f32),
            sp2,
        ]
    )                                                   # [3, H]
    cstP = np.ascontiguousarray(
        cstv.reshape(3, NJ, 128).transpose(2, 0, 1).reshape(128, 3 * NJ)
    ).astype(f32)
    ndt2 = (np.asarray(delta_t, f32) * -0.5).astype(np.float16)   # [B]

    CC = KT * NCHUNK
    in_maps = []
    for c in range(NCORES):
        sl = slice(c * BS, (c + 1) * BS)
        xgp = _pack_cmajor(xg8[:, sl], KT, NCH, NCHUNK)
        xhp = _pack_cmajor(xh16[:, sl], KT, NCH, NCHUNK)
        in_maps.append(
            {
                "gc8": np.concatenate([w8g, xgp[:, 0:CC]], axis=1),
                "xgc1": np.ascontiguousarray(xgp[:, CC:2 * CC]),
                "bc16": np.concatenate([w16b, xhp[:, 0:CC]], axis=1),
                "xhc1": np.ascontiguousarray(xhp[:, CC:2 * CC]),
                "ndt2": np.ascontiguousarray(
                    np.broadcast_to(ndt2[sl][None, :], (128, BS))
                ),
                "consts": cstP,
            }
        )
    return in_maps


def kernel(x, h, delta_t, W_backbone, b_backbone, W_gx, b_gx, W_gh, gate_b,
           log_tau):
    global LAST_RESULT
    in_maps = make_in_maps(x, h, delta_t, W_backbone, b_backbone, W_gx, b_gx,
                           W_gh, gate_b, log_tau)
    nc = _get_nc()
    res = run_bass_kernel_spmd(
        nc, in_maps, core_ids=list(range(NCORES)), trace=TRACE
    )
    LAST_RESULT = res
    # outP is [128, NJ*BS] partition-major; unpack to [H, BS] then gather.
    outs = []
    for r in res.results:
        op = r["outP"].reshape(128, NJ, BS).transpose(1, 0, 2).reshape(H, BS)
        outs.append(op)
    out = np.concatenate(outs, axis=1).T
    return np.ascontiguousarray(out).astype(np.float32)


# revision 37
# speedup vs baseline: 1.1075x; 1.1075x over previous
"""CfC (closed-form continuous-time) cell kernel for Trainium2, 8 NeuronCores.

Reference computation (B=8192, IN=256, H=512, all fp32):
    g     = sigmoid(x @ W_gx.T + b_gx + h @ W_gh.T + gate_b)        [B, H]
    f     = tanh(cat([x, h]) @ W_backbone.T + b_backbone)           [B, H]
    tau   = softplus(log_tau) + |g|          (g in (0,1) so |g| == g)
    decay = exp(-delta_t[:, None] * tau)
    out   = decay * h + (1 - decay) * f

Strategy: data-parallel over B (1024 rows per core), weights replicated.
Feature-major on device: activations ship as xh^T [768, B_shard] so the
contraction dim lands on SBUF partitions with no on-device transposes.

Matmul precision split: the GATE matmul runs in fp8-e4m3 DoubleRow perf
mode (2x PE rate; the gate feeds sigmoid -> tau -> exp, so its error is
strongly attenuated), while the BACKBONE matmul (direct path to the output)
stays fp16.  Gate operands are pre-scaled by powers of two (x*8, W*32) to
center them in e4m3's normal range; the 1/256 dequant plus the sigmoid
half-angle 1/2 fold into the ACT scale (1/512).

Two-phase schedule, sized by the input-DMA roofline (~3.7MB at ~300GB/s):
the gate phase needs only the small fp8 streams (1.15MB) so it starts as
soon as they land; its epilogue chain (tg -> t -> decay) has no backbone
dependency.  The backbone phase (fp16, 2.3MB) streams in behind it.  Both
phases emit matmuls j-tile by j-tile through a single 4-deep PSUM ring.

    phase A, per j:  zg = DoubleRow-fp8 matmuls
                     tg    = Tanh(zg/512 + bg/2)              ACT
                     t     = (tg + 2*softplus'+1) * (-dt/2)   DVE STT
                     decay = Exp(t)                           ACT
    phase B, per j:  zf = fp16 matmuls
                     f     = Tanh(zf + bb)                    ACT
                     hmf   = h - f                            DVE
                     p     = decay * hmf                      DVE
                     o     = p + f -> DMA out (fp16)          DVE

softplus(log_tau) is a [H] constant computed on the host, so the scalar
engine needs one ACT table set (tanh+exp) loaded once.  All 16-bit DVE ops
hit the 2x packed mode.  Activations are packed chunk-major so each DMA is
one contiguous-row panel in matmul consumption order; triggers are split
across the two HWDGE rings (sync: gate stream, scalar: backbone stream)
plus gpsimd SWDGE for -dt/2.  The last j-tile runs a chunk-split epilogue
to shorten the post-matmul tail.  Output ships fp16, upcast on the host.
"""

from contextlib import ExitStack

import ml_dtypes
import numpy as np

import concourse.bass as bass
import concourse.mybir as mybir
import concourse.tile as tile
from concourse import bacc
from concourse.bass_utils import run_bass_kernel_spmd

B, IN, H = 8192, 256, 512
NCORES = 8
BS = B // NCORES          # 1024 batch rows per core
KIN = IN + H              # 768 contraction dim
KT = KIN // 128           # 6 k-tiles
NP = KT // 2              # 3 k-pair panels (DoubleRow processes 2 k-tiles)
NJ = H // 128             # 4 partition tiles per output matrix
NCHUNK = 512              # matmul moving free dim per PSUM bank
NCH = BS // NCHUNK        # 2 b-chunks per core

SX = 8.0                  # gate activation pre-scale (power of 2)
SW = 32.0                 # gate weight pre-scale (power of 2)
GDEQ = 1.0 / (SX * SW * 2.0)   # ACT scale: dequant + sigmoid half-angle

F32 = mybir.dt.float32
FP16 = mybir.dt.float16
FP8 = mybir.dt.float8e4
AF = mybir.ActivationFunctionType
OP = mybir.AluOpType
PM = mybir.MatmulPerfMode

TRACE = False             # test.py flips this for profiled runs
LAST_RESULT = None        # BassKernelResults of the most recent run

_NC_CACHE = None


def _body(tc, gc8, xgc1, bc16, xhc1, ndt2, consts, outP):
    nc = tc.nc
    with ExitStack() as ctx:
        singles = ctx.enter_context(tc.tile_pool(name="singles", bufs=1))
        decs = ctx.enter_context(tc.tile_pool(name="decs", bufs=2))
        work = ctx.enter_context(tc.tile_pool(name="work", bufs=2))
        psg = ctx.enter_context(tc.tile_pool(name="psg", bufs=2, space="PSUM"))
        psf = ctx.enter_context(tc.tile_pool(name="psf", bufs=2, space="PSUM"))

        # Persistent SBUF tensors.  Activation streams are chunk-panel
        # tiles so each DMA is one contiguous-row panel in matmul
        # consumption order.
        wg_sb = singles.tile([128, KT, H], FP8, tag="wg")
        xg0_sb = singles.tile([128, KT, NCHUNK], FP8, tag="xg0")
        xg1_sb = singles.tile([128, KT, NCHUNK], FP8, tag="xg1")
        wb_sb = singles.tile([128, KT, H], FP16, tag="wb")
        xh0_sb = singles.tile([128, KT, NCHUNK], FP16, tag="xh0")
        xh1_sb = singles.tile([128, KT, NCHUNK], FP16, tag="xh1")
        ndt_sb = singles.tile([128, BS], FP16, tag="ndt")
        cst = singles.tile([128, 3, NJ], F32, tag="cst")

        wg_v = wg_sb
        xg_v = {0: xg0_sb, 1: xg1_sb}           # per-chunk moving panels
        wb_v = wb_sb
        xh_v = {0: xh0_sb, 1: xh1_sb}

        CC = KT * NCHUNK
        # All input DMAs ride ONE HWDGE ring (sync) in exact consumption
        # order: the SDMA engines round-robin between rings at packet
        # granularity, so two concurrent rings starve whichever stream is
        # needed first.  One ring drains FIFO at full fabric rate.
        nc.sync.dma_start(out=wg_sb.rearrange("p k n -> p (k n)"),
                          in_=gc8[:, 0:CC])
        nc.sync.dma_start(out=xg0_sb.rearrange("p k c -> p (k c)"),
                          in_=gc8[:, CC:2 * CC])
        nc.sync.dma_start(out=xg1_sb.rearrange("p k c -> p (k c)"),
                          in_=xgc1)
        nc.sync.dma_start(out=wb_sb.rearrange("p k n -> p (k n)"),
                          in_=bc16[:, 0:CC])
        nc.sync.dma_start(out=xh0_sb.rearrange("p k c -> p (k c)"),
                          in_=bc16[:, CC:2 * CC])
        nc.sync.dma_start(out=xh1_sb.rearrange("p k c -> p (k c)"),
                          in_=xhc1)
        # Epilogue constants + -dt/2 broadcast ride the gpsimd SWDGE ring
        # (48B rows would clog the head of the main input ring).
        nc.gpsimd.dma_start(out=cst.rearrange("p c j -> p (c j)"),
                            in_=consts)
        nc.gpsimd.dma_start(out=ndt_sb, in_=ndt2)

        # --- Phase A: gate matmuls + gate chain (tg -> t -> decay) ---
        # t for j-pairs lands in one [128, 2*BS] tile so Exp runs as two
        # double-width ACTs (halves the per-instruction ACT overhead).
        # The otherwise-idle DVE then precomputes em = 1-decay and
        # dech = decay*h, so each phase-B combine is only two hops after
        # its tanh: o = f*em + dech.
        em = {}
        dech = {}
        tp = {}
        for j in range(NJ):
            zg = psg.tile([128, BS], F32, tag="zg", name=f"zg_{j}")
            for n in range(NCH):
                bsl = slice(n * NCHUNK, (n + 1) * NCHUNK)
                for g in range(NP):
                    nc.tensor.matmul(
                        zg[:, bsl],
                        wg_v[:, 2 * g:2 * g + 2, j * 128:(j + 1) * 128],
                        xg_v[n][:, 2 * g:2 * g + 2, :],
                        start=(g == 0),
                        stop=(g == NP - 1),
                        perf_mode=PM.DoubleRow,
                    )
            tg = work.tile([128, BS], FP16, tag="tg", name=f"tg_{j}")
            if j % 2 == 0:
                tp[j // 2] = work.tile([128, 2 * BS], FP16, tag="t",
                                       name=f"t_{j // 2}")
            # tg = tanh((zg + 256*bg)/512) = tanh(zg_true/2 + bg/2)
            nc.scalar.activation(
                out=tg, in_=zg, func=AF.Tanh, bias=cst[:, 0, j:j + 1],
                scale=GDEQ,
            )
            # t = (tg + (2*softplus+1)) * (-dt/2)  [= -dt * (softplus + g)]
            nc.vector.scalar_tensor_tensor(
                out=tp[j // 2][:, (j % 2) * BS:(j % 2 + 1) * BS],
                in0=tg, scalar=cst[:, 2, j:j + 1], in1=ndt_sb,
                op0=OP.add, op1=OP.mult,
            )
            if j % 2 == 1:
                dp = decs.tile([128, 2 * BS], FP16, tag="dec",
                               name=f"dec_{j // 2}")
                nc.scalar.activation(out=dp, in_=tp[j // 2], func=AF.Exp)
                ep = decs.tile([128, 2 * BS], FP16, tag="em",
                               name=f"em_{j // 2}")
                nc.vector.tensor_scalar(
                    out=ep, in0=dp, scalar1=-1.0, scalar2=1.0,
                    op0=OP.mult, op1=OP.add,
                )
                for jj in (j - 1, j):
                    em[jj] = ep[:, (jj % 2) * BS:(jj % 2 + 1) * BS]
                    dh = decs.tile([128, BS], FP16, tag="dech",
                                   name=f"dech_{jj}")
                    for c in range(NCH):
                        nc.vector.tensor_mul(
                            out=dh[:, c * NCHUNK:(c + 1) * NCHUNK],
                            in0=dp[:, (jj % 2) * BS + c * NCHUNK:
                                   (jj % 2) * BS + (c + 1) * NCHUNK],
                            in1=xh_v[c][:, 2 + jj, :],
                        )
                    dech[jj] = dh

        # --- Phase B: backbone matmuls + f + combine ---
        # Full-width combines for j<3; the last j-tile splits its epilogue
        # into one 512 chunk and two 256 half-chunks so every hop on the
        # final critical path gets cheaper.
        def combine(j, zf, csl, name):
            w = csl.stop - csl.start
            f = work.tile([128, w], FP16, tag=f"f{w}", name=f"f_{name}")
            p = work.tile([128, w], FP16, tag=f"p{w}", name=f"p_{name}")
            o = work.tile([128, w], FP16, tag=f"o{w}", name=f"o_{name}")
            nc.scalar.activation(
                out=f, in_=zf[:, csl], func=AF.Tanh, bias=cst[:, 1, j:j + 1]
            )
            # o = f*(1-decay) + decay*h, both factors precomputed in phase A
            nc.vector.tensor_mul(out=p, in0=f, in1=em[j][:, csl])
            nc.vector.tensor_add(out=o, in0=p, in1=dech[j][:, csl])
            nc.sync.dma_start(
                out=outP[:, j * BS + csl.start:j * BS + csl.stop], in_=o
            )

        for j in range(NJ):
            zf = psf.tile([128, BS], F32, tag="zf", name=f"zf_{j}")
            for n in range(NCH):
                bsl = slice(n * NCHUNK, (n + 1) * NCHUNK)
                for k in range(KT):
                    nc.tensor.matmul(
                        zf[:, bsl],
                        wb_v[:, k, j * 128:(j + 1) * 128],
                        xh_v[n][:, k, :],
                        start=(k == 0),
                        stop=(k == KT - 1),
                    )
            if j < NJ - 1:
                combine(j, zf, slice(0, BS), f"{j}")
            else:
                # Last tile: one full-width tanh, then quarter-width V
                # chains + DMAs so the final hops are as cheap as possible.
                f = work.tile([128, BS], FP16, tag="f1024", name="f_3")
                nc.scalar.activation(
                    out=f, in_=zf, func=AF.Tanh, bias=cst[:, 1, j:j + 1]
                )
                for q in range(2):
                    csl = slice(q * (BS // 2), (q + 1) * (BS // 2))
                    w = BS // 2
                    p = work.tile([128, w], FP16, tag="pq", name=f"p_3_{q}")
                    o = work.tile([128, w], FP16, tag="oq", name=f"o_3_{q}")
                    nc.vector.tensor_mul(out=p, in0=f[:, csl],
                                         in1=em[j][:, csl])
                    nc.vector.tensor_add(out=o, in0=p, in1=dech[j][:, csl])
                    nc.sync.dma_start(
                        out=outP[:, j * BS + csl.start:j * BS + csl.stop],
                        in_=o
                    )


def build_nc():
    nc = bacc.Bacc(
        "TRN2",
        target_bir_lowering=False,
        debug=False,
        enable_asserts=False,
        num_devices=NCORES,
    )
    # Partition-major packed streams: row p holds that partition's entire
    # contiguous payload.
    CC = KT * NCHUNK
    gc8 = nc.dram_tensor("gc8", [128, 2 * CC], FP8, kind="ExternalInput").ap()
    xgc1 = nc.dram_tensor("xgc1", [128, CC], FP8, kind="ExternalInput").ap()
    bc16 = nc.dram_tensor("bc16", [128, 2 * CC], FP16, kind="ExternalInput").ap()
    xhc1 = nc.dram_tensor("xhc1", [128, CC], FP16, kind="ExternalInput").ap()
    ndt2 = nc.dram_tensor("ndt2", [128, BS], FP16, kind="ExternalInput").ap()
    consts = nc.dram_tensor("consts", [128, 3 * NJ], F32,
                            kind="ExternalInput").ap()
    outP = nc.dram_tensor("outP", [128, NJ * BS], FP16, kind="ExternalOutput").ap()
    with tile.TileContext(nc) as tc:
        _body(tc, gc8, xgc1, bc16, xhc1, ndt2, consts, outP)
    nc.compile()
    return nc


def _get_nc():
    global _NC_CACHE
    if _NC_CACHE is None:
        _NC_CACHE = build_nc()
    return _NC_CACHE


def _pack_cmajor(a, kt, nch, nchunk):
    """[kt*128, nch*nchunk] -> [128, nch*kt*nchunk] chunk-major pack: row p
    holds [chunk0: k0..k5 | chunk1: k0..k5], each 128-partition-sliced."""
    return np.ascontiguousarray(
        a.reshape(kt, 128, nch, nchunk).transpose(1, 2, 0, 3)
        .reshape(128, nch * kt * nchunk)
    )


def _pack_pmajor(a, kt):
    """[kt*128, C] -> [128, kt*C]: partition-major pack so each of the 128
    DMA rows is contiguous in DRAM."""
    c = a.shape[1]
    return np.ascontiguousarray(
        a.reshape(kt, 128, c).transpose(1, 0, 2).reshape(128, kt * c)
    )


def make_in_maps(x, h, delta_t, W_backbone, b_backbone, W_gx, b_gx, W_gh,
                 gate_b, log_tau):
    f32 = np.float32
    xh = np.concatenate(
        [np.asarray(x, f32), np.asarray(h, f32)], axis=1
    )                                                   # [B, 768]
    xhT = np.ascontiguousarray(xh.T)                    # [768, B] f32
    xh16 = xhT.astype(np.float16)
    xg8 = np.asarray(xhT * SX, dtype=ml_dtypes.float8_e4m3)

    WgT = np.concatenate(
        [np.asarray(W_gx, f32), np.asarray(W_gh, f32)], axis=1
    ).T                                                 # [768, H]
    w8g = _pack_pmajor(np.asarray(WgT * SW, dtype=ml_dtypes.float8_e4m3), KT)
    w16b = _pack_pmajor(
        np.ascontiguousarray(np.asarray(W_backbone, f32).T).astype(np.float16),
        KT,
    )

    sp2 = 2.0 * np.log1p(np.exp(np.asarray(log_tau, f32))) + 1.0
    # cstP[p, c*NJ+j] = const_c[j*128+p]
    cstv = np.stack(
        [
            (np.asarray(b_gx, f32) + np.asarray(gate_b, f32)) * 0.5,
            np.asarray(b_backbone, f32),
            sp2,
        ]
    )                                                   # [3, H]
    cstP = np.ascontiguousarray(
        cstv.reshape(3, NJ, 128).transpose(2, 0, 1).reshape(128, 3 * NJ)
    ).astype(f32)
    ndt2 = (np.asarray(delta_t, f32) * -0.5).astype(np.float16)   # [B]

    CC = KT * NCHUNK
    in_maps = []
    for c in range(NCORES):
        sl = slice(c * BS, (c + 1) * BS)
        xgp = _pack_cmajor(xg8[:, sl], KT, NCH, NCHUNK)
        xhp = _pack_cmajor(xh16[:, sl], KT, NCH, NCHUNK)
        in_maps.append(
            {
                "gc8": np.concatenate([w8g, xgp[:, 0:CC]], axis=1),
                "xgc1": np.ascontiguousarray(xgp[:, CC:2 * CC]),
                "bc16": np.concatenate([w16b, xhp[:, 0:CC]], axis=1),
                "xhc1": np.ascontiguousarray(xhp[:, CC:2 * CC]),
                "ndt2": np.ascontiguousarray(
                    np.broadcast_to(ndt2[sl][None, :], (128, BS))
                ),
                "consts": cstP,
            }
        )
    return in_maps


def kernel(x, h, delta_t, W_backbone, b_backbone, W_gx, b_gx, W_gh, gate_b,
           log_tau):
    global LAST_RESULT
    in_maps = make_in_maps(x, h, delta_t, W_backbone, b_backbone, W_gx, b_gx,
                           W_gh, gate_b, log_tau)
    nc = _get_nc()
    res = run_bass_kernel_spmd(
        nc, in_maps, core_ids=list(range(NCORES)), trace=TRACE
    )
    LAST_RESULT = res
    # outP is [128, NJ*BS] partition-major; unpack to [H, BS] then gather.
    outs = []
    for r in res.results:
        op = r["outP"].reshape(128, NJ, BS).transpose(1, 0, 2).reshape(H, BS)
        outs.append(op)
    out = np.concatenate(outs, axis=1).T
    return np.ascontiguousarray(out).astype(np.float32)


# revision 39
# speedup vs baseline: 1.1159x; 1.0076x over previous
"""CfC (closed-form continuous-time) cell kernel for Trainium2, 8 NeuronCores.

Reference computation (B=8192, IN=256, H=512, all fp32):
    g     = sigmoid(x @ W_gx.T + b_gx + h @ W_gh.T + gate_b)        [B, H]
    f     = tanh(cat([x, h]) @ W_backbone.T + b_backbone)           [B, H]
    tau   = softplus(log_tau) + |g|          (g in (0,1) so |g| == g)
    decay = exp(-delta_t[:, None] * tau)
    out   = decay * h + (1 - decay) * f

Strategy: data-parallel over B (1024 rows per core), weights replicated.
Feature-major on device: activations ship as xh^T [768, B_shard] so the
contraction dim lands on SBUF partitions with no on-device transposes.

Matmul precision split: the GATE matmul runs in fp8-e4m3 DoubleRow perf
mode (2x PE rate; the gate feeds sigmoid -> tau -> exp, so its error is
strongly attenuated), while the BACKBONE matmul (direct path to the output)
stays fp16.  Gate operands are pre-scaled by powers of two (x*8, W*32) to
center them in e4m3's normal range; the 1/256 dequant plus the sigmoid
half-angle 1/2 fold into the ACT scale (1/512).

Two-phase schedule, sized by the input-DMA roofline (~3.5MB at ~300GB/s):
the gate phase needs only the small fp8 streams (1.15MB) so it starts as
soon as they land; its chain (tg -> t -> decay) has no backbone
dependency.  The backbone phase (fp16, 2.3MB) streams in behind it.

    phase A, per j:  zg = DoubleRow-fp8 matmuls
                     tg    = Tanh(zg/512 + bg/2)              ACT
                     t     = (tg + 2*softplus'+1) * (-dt/2)   DVE STT
                     decay = Exp(t)       (ACT, one per j-pair, 2x wide)
                     em    = 1 - decay                        DVE TS
                     dech  = decay * h                        DVE
    phase B, per j:  zf = fp16 matmuls
                     f     = Tanh(zf + bb)                    ACT
                     o     = f * em + dech -> DMA out (fp16)  DVE x2

em/dech are precomputed on the otherwise-idle DVE during phase A so each
phase-B combine is only two DVE hops after its tanh — the post-last-matmul
tail stays short (the last j-tile further splits its combine in halves).
softplus(log_tau) is a [H] constant computed on the host, so the scalar
engine needs one ACT table set (tanh+exp) loaded once.  All 16-bit DVE ops
hit the 2x packed mode.

All input DMAs ride ONE HWDGE ring (sync) in exact consumption order —
concurrent rings round-robin per packet and starve whichever stream is
needed first — with small fp8 panels first so the first matmul's data
(768KB) lands as early as possible.  Tiny constants ride the gpsimd SWDGE
ring.  Output ships fp16 and is upcast on the host.
"""

from contextlib import ExitStack

import ml_dtypes
import numpy as np

import concourse.mybir as mybir
import concourse.tile as tile
from concourse import bacc
from concourse.bass_utils import run_bass_kernel_spmd

B, IN, H = 8192, 256, 512
NCORES = 8
BS = B // NCORES          # 1024 batch rows per core
KIN = IN + H              # 768 contraction dim
KT = KIN // 128           # 6 k-tiles
NP = KT // 2              # 3 k-pair panels (DoubleRow processes 2 k-tiles)
NJ = H // 128             # 4 partition tiles per output matrix
NCHUNK = 512              # matmul moving free dim per PSUM bank
NCH = BS // NCHUNK        # 2 b-chunks per core

SX = 8.0                  # gate activation pre-scale (power of 2)
SW = 32.0                 # gate weight pre-scale (power of 2)
GDEQ = 1.0 / (SX * SW * 2.0)   # ACT scale: dequant + sigmoid half-angle

F32 = mybir.dt.float32
FP16 = mybir.dt.float16
FP8 = mybir.dt.float8e4
AF = mybir.ActivationFunctionType
OP = mybir.AluOpType
PM = mybir.MatmulPerfMode

TRACE = False             # test.py flips this for profiled runs
LAST_RESULT = None        # BassKernelResults of the most recent run

_NC_CACHE = None


def _body(tc, gc8, xgc1, bc16, xhc1, ndt2, consts, outP):
    nc = tc.nc
    with ExitStack() as ctx:
        singles = ctx.enter_context(tc.tile_pool(name="singles", bufs=1))
        decs = ctx.enter_context(tc.tile_pool(name="decs", bufs=2))
        work = ctx.enter_context(tc.tile_pool(name="work", bufs=2))
        psg = ctx.enter_context(tc.tile_pool(name="psg", bufs=2, space="PSUM"))
        psf = ctx.enter_context(tc.tile_pool(name="psf", bufs=2, space="PSUM"))

        # Persistent SBUF tensors.  Activation streams are chunk-panel
        # tiles so each DMA is one contiguous-row panel in matmul
        # consumption order.
        wg_sb = singles.tile([128, KT, H], FP8, tag="wg")
        xg0_sb = singles.tile([128, KT, NCHUNK], FP8, tag="xg0")
        xg1_sb = singles.tile([128, KT, NCHUNK], FP8, tag="xg1")
        wb_sb = singles.tile([128, KT, H], FP16, tag="wb")
        xh0_sb = singles.tile([128, KT, NCHUNK], FP16, tag="xh0")
        xh1_sb = singles.tile([128, KT, NCHUNK], FP16, tag="xh1")
        ndt_sb = singles.tile([128, BS], FP16, tag="ndt")
        cst = singles.tile([128, 3, NJ], F32, tag="cst")

        wg_v = wg_sb
        xg_v = {0: xg0_sb, 1: xg1_sb}           # per-chunk moving panels
        wb_v = wb_sb
        xh_v = {0: xh0_sb, 1: xh1_sb}

        CC = KT * NCHUNK
        # All input DMAs ride ONE HWDGE ring (sync) in exact consumption
        # order: the SDMA engines round-robin between rings at packet
        # granularity, so two concurrent rings starve whichever stream is
        # needed first.  One ring drains FIFO at full fabric rate.
        nc.sync.dma_start(out=wg_sb.rearrange("p k n -> p (k n)"),
                          in_=gc8[:, 0:CC])
        nc.sync.dma_start(out=xg0_sb.rearrange("p k c -> p (k c)"),
                          in_=gc8[:, CC:2 * CC])
        nc.sync.dma_start(out=xg1_sb.rearrange("p k c -> p (k c)"),
                          in_=xgc1)
        nc.sync.dma_start(out=wb_sb.rearrange("p k n -> p (k n)"),
                          in_=bc16[:, 0:CC])
        nc.sync.dma_start(out=xh0_sb.rearrange("p k c -> p (k c)"),
                          in_=bc16[:, CC:2 * CC])
        nc.sync.dma_start(out=xh1_sb.rearrange("p k c -> p (k c)"),
                          in_=xhc1)
        # Epilogue constants + -dt/2 broadcast ride the gpsimd SWDGE ring
        # (48B rows would clog the head of the main input ring).
        nc.gpsimd.dma_start(out=cst.rearrange("p c j -> p (c j)"),
                            in_=consts)
        nc.gpsimd.dma_start(out=ndt_sb, in_=ndt2)

        # --- Phase A: gate matmuls + gate chain (tg -> t -> decay) ---
        # t for j-pairs lands in one [128, 2*BS] tile so Exp runs as two
        # double-width ACTs (halves the per-instruction ACT overhead).
        # The otherwise-idle DVE then precomputes em = 1-decay and
        # dech = decay*h, so each phase-B combine is only two hops after
        # its tanh: o = f*em + dech.
        em = {}
        dech = {}
        tp = {}
        for j in range(NJ):
            zg = psg.tile([128, BS], F32, tag="zg", name=f"zg_{j}")
            for n in range(NCH):
                bsl = slice(n * NCHUNK, (n + 1) * NCHUNK)
                for g in range(NP):
                    nc.tensor.matmul(
                        zg[:, bsl],
                        wg_v[:, 2 * g:2 * g + 2, j * 128:(j + 1) * 128],
                        xg_v[n][:, 2 * g:2 * g + 2, :],
                        start=(g == 0),
                        stop=(g == NP - 1),
                        perf_mode=PM.DoubleRow,
                    )
            tg = work.tile([128, BS], FP16, tag="tg", name=f"tg_{j}")
            if j % 2 == 0:
                tp[j // 2] = work.tile([128, 2 * BS], FP16, tag="t",
                                       name=f"t_{j // 2}")
            # tg = tanh((zg + 256*bg)/512) = tanh(zg_true/2 + bg/2)
            nc.scalar.activation(
                out=tg, in_=zg, func=AF.Tanh, bias=cst[:, 0, j:j + 1],
                scale=GDEQ,
            )
            # t = (tg + (2*softplus+1)) * (-dt/2)  [= -dt * (softplus + g)]
            nc.vector.scalar_tensor_tensor(
                out=tp[j // 2][:, (j % 2) * BS:(j % 2 + 1) * BS],
                in0=tg, scalar=cst[:, 2, j:j + 1], in1=ndt_sb,
                op0=OP.add, op1=OP.mult,
            )
            if j % 2 == 1:
                dp = decs.tile([128, 2 * BS], FP16, tag="dec",
                               name=f"dec_{j // 2}")
                nc.scalar.activation(out=dp, in_=tp[j // 2], func=AF.Exp)
                ep = decs.tile([128, 2 * BS], FP16, tag="em",
                               name=f"em_{j // 2}")
                nc.vector.tensor_scalar(
                    out=ep, in0=dp, scalar1=-1.0, scalar2=1.0,
                    op0=OP.mult, op1=OP.add,
                )
                for jj in (j - 1, j):
                    em[jj] = ep[:, (jj % 2) * BS:(jj % 2 + 1) * BS]
                    dh = decs.tile([128, BS], FP16, tag="dech",
                                   name=f"dech_{jj}")
                    for c in range(NCH):
                        nc.vector.tensor_mul(
                            out=dh[:, c * NCHUNK:(c + 1) * NCHUNK],
                            in0=dp[:, (jj % 2) * BS + c * NCHUNK:
                                   (jj % 2) * BS + (c + 1) * NCHUNK],
                            in1=xh_v[c][:, 2 + jj, :],
                        )
                    dech[jj] = dh

        # --- Phase B: backbone matmuls + f + combine ---
        # Full-width combines for j<3; the last j-tile splits its epilogue
        # into one 512 chunk and two 256 half-chunks so every hop on the
        # final critical path gets cheaper.
        def combine(j, zf, csl, name):
            w = csl.stop - csl.start
            f = work.tile([128, w], FP16, tag=f"f{w}", name=f"f_{name}")
            p = work.tile([128, w], FP16, tag=f"p{w}", name=f"p_{name}")
            o = work.tile([128, w], FP16, tag=f"o{w}", name=f"o_{name}")
            nc.scalar.activation(
                out=f, in_=zf[:, csl], func=AF.Tanh, bias=cst[:, 1, j:j + 1]
            )
            # o = f*(1-decay) + decay*h, both factors precomputed in phase A
            nc.vector.tensor_mul(out=p, in0=f, in1=em[j][:, csl])
            nc.vector.tensor_add(out=o, in0=p, in1=dech[j][:, csl])
            nc.sync.dma_start(
                out=outP[:, j * BS + csl.start:j * BS + csl.stop], in_=o
            )

        for j in range(NJ):
            zf = psf.tile([128, BS], F32, tag="zf", name=f"zf_{j}")
            for n in range(NCH):
                bsl = slice(n * NCHUNK, (n + 1) * NCHUNK)
                for k in range(KT):
                    nc.tensor.matmul(
                        zf[:, bsl],
                        wb_v[:, k, j * 128:(j + 1) * 128],
                        xh_v[n][:, k, :],
                        start=(k == 0),
                        stop=(k == KT - 1),
                    )
            if j < NJ - 1:
                combine(j, zf, slice(0, BS), f"{j}")
            else:
                # Last tile: one full-width tanh, then quarter-width V
                # chains + DMAs so the final hops are as cheap as possible.
                f = work.tile([128, BS], FP16, tag="f1024", name="f_3")
                nc.scalar.activation(
                    out=f, in_=zf, func=AF.Tanh, bias=cst[:, 1, j:j + 1]
                )
                for q in range(2):
                    csl = slice(q * (BS // 2), (q + 1) * (BS // 2))
                    w = BS // 2
                    p = work.tile([128, w], FP16, tag="pq", name=f"p_3_{q}")
                    o = work.tile([128, w], FP16, tag="oq", name=f"o_3_{q}")
                    nc.vector.tensor_mul(out=p, in0=f[:, csl],
                                         in1=em[j][:, csl])
                    nc.vector.tensor_add(out=o, in0=p, in1=dech[j][:, csl])
                    nc.sync.dma_start(
                        out=outP[:, j * BS + csl.start:j * BS + csl.stop],
                        in_=o
                    )


def build_nc():
    nc = bacc.Bacc(
        "TRN2",
        target_bir_lowering=False,
        debug=False,
        enable_asserts=False,
        num_devices=NCORES,
    )
    # Partition-major packed streams: row p holds that partition's entire
    # contiguous payload.
    CC = KT * NCHUNK
    gc8 = nc.dram_tensor("gc8", [128, 2 * CC], FP8, kind="ExternalInput").ap()
    xgc1 = nc.dram_tensor("xgc1", [128, CC], FP8, kind="ExternalInput").ap()
    bc16 = nc.dram_tensor("bc16", [128, 2 * CC], FP16, kind="ExternalInput").ap()
    xhc1 = nc.dram_tensor("xhc1", [128, CC], FP16, kind="ExternalInput").ap()
    ndt2 = nc.dram_tensor("ndt2", [128, BS], FP16, kind="ExternalInput").ap()
    consts = nc.dram_tensor("consts", [128, 3 * NJ], F32,
                            kind="ExternalInput").ap()
    outP = nc.dram_tensor("outP", [128, NJ * BS], FP16, kind="ExternalOutput").ap()
    with tile.TileContext(nc) as tc:
        _body(tc, gc8, xgc1, bc16, xhc1, ndt2, consts, outP)
    nc.compile()
    return nc


def _get_nc():
    global _NC_CACHE
    if _NC_CACHE is None:
        _NC_CACHE = build_nc()
    return _NC_CACHE


def _pack_cmajor(a, kt, nch, nchunk):
    """[kt*128, nch*nchunk] -> [128, nch*kt*nchunk] chunk-major pack: row p
    holds [chunk0: k0..k5 | chunk1: k0..k5], each 128-partition-sliced."""
    return np.ascontiguousarray(
        a.reshape(kt, 128, nch, nchunk).transpose(1, 2, 0, 3)
        .reshape(128, nch * kt * nchunk)
    )


def _pack_pmajor(a, kt):
    """[kt*128, C] -> [128, kt*C]: partition-major pack so each of the 128
    DMA rows is contiguous in DRAM."""
    c = a.shape[1]
    return np.ascontiguousarray(
        a.reshape(kt, 128, c).transpose(1, 0, 2).reshape(128, kt * c)
    )


def make_in_maps(x, h, delta_t, W_backbone, b_backbone, W_gx, b_gx, W_gh,
                 gate_b, log_tau):
    f32 = np.float32
    xh = np.concatenate(
        [np.asarray(x, f32), np.asarray(h, f32)], axis=1
    )                                                   # [B, 768]
    xhT = np.ascontiguousarray(xh.T)                    # [768, B] f32
    xh16 = xhT.astype(np.float16)
    xg8 = np.asarray(xhT * SX, dtype=ml_dtypes.float8_e4m3)

    WgT = np.concatenate(
        [np.asarray(W_gx, f32), np.asarray(W_gh, f32)], axis=1
    ).T                                                 # [768, H]
    w8g = _pack_pmajor(np.asarray(WgT * SW, dtype=ml_dtypes.float8_e4m3), KT)
    w16b = _pack_pmajor(
        np.ascontiguousarray(np.asarray(W_backbone, f32).T).astype(np.float16),
        KT,
    )

    sp2 = 2.0 * np.log1p(np.exp(np.asarray(log_tau, f32))) + 1.0
    # cstP[p, c*NJ+j] = const_c[j*128+p]
    cstv = np.stack(
        [
            (np.asarray(b_gx, f32) + np.asarray(gate_b, f32)) * 0.5,
            np.asarray(b_backbone, f32),
            sp2,
        ]
    )                                                   # [3, H]
    cstP = np.ascontiguousarray(
        cstv.reshape(3, NJ, 128).transpose(2, 0, 1).reshape(128, 3 * NJ)
    ).astype(f32)
    ndt2 = (np.asarray(delta_t, f32) * -0.5).astype(np.float16)   # [B]

    CC = KT * NCHUNK
    in_maps = []
    for c in range(NCORES):
        sl = slice(c * BS, (c + 1) * BS)
        xgp = _pack_cmajor(xg8[:, sl], KT, NCH, NCHUNK)
        xhp = _pack_cmajor(xh16[:, sl], KT, NCH, NCHUNK)
        in_maps.append(
            {
                "gc8": np.concatenate([w8g, xgp[:, 0:CC]], axis=1),
                "xgc1": np.ascontiguousarray(xgp[:, CC:2 * CC]),
                "bc16": np.concatenate([w16b, xhp[:, 0:CC]], axis=1),
                "xhc1": np.ascontiguousarray(xhp[:, CC:2 * CC]),
                "ndt2": np.ascontiguousarray(
                    np.broadcast_to(ndt2[sl][None, :], (128, BS))
                ),
                "consts": cstP,
            }
        )
    return in_maps


def kernel(x, h, delta_t, W_backbone, b_backbone, W_gx, b_gx, W_gh, gate_b,
           log_tau):
    global LAST_RESULT
    in_maps = make_in_maps(x, h, delta_t, W_backbone, b_backbone, W_gx, b_gx,
                           W_gh, gate_b, log_tau)
    nc = _get_nc()
    res = run_bass_kernel_spmd(
        nc, in_maps, core_ids=list(range(NCORES)), trace=TRACE
    )
    LAST_RESULT = res
    # outP is [128, NJ*BS] partition-major; unpack to [H, BS] then gather.
    outs = []
    for r in res.results:
        op = r["outP"].reshape(128, NJ, BS).transpose(1, 0, 2).reshape(H, BS)
        outs.append(op)
    out = np.concatenate(outs, axis=1).T
    return np.ascontiguousarray(out).astype(np.float32)


# revision 43
# speedup vs baseline: 1.1493x; 1.0300x over previous
"""CfC (closed-form continuous-time) cell kernel for Trainium2, 8 NeuronCores.

Reference computation (B=8192, IN=256, H=512, all fp32):
    g     = sigmoid(x @ W_gx.T + b_gx + h @ W_gh.T + gate_b)        [B, H]
    f     = tanh(cat([x, h]) @ W_backbone.T + b_backbone)           [B, H]
    tau   = softplus(log_tau) + |g|          (g in (0,1) so |g| == g)
    decay = exp(-delta_t[:, None] * tau)
    out   = decay * h + (1 - decay) * f

Strategy: data-parallel over B (1024 rows per core), weights replicated.
Feature-major on device: activations ship as xh^T [768, B_shard] so the
contraction dim lands on SBUF partitions with no on-device transposes.

Matmul precision split: the GATE matmul runs in fp8-e4m3 DoubleRow perf
mode (2x PE rate; the gate feeds sigmoid -> tau -> exp, so its error is
strongly attenuated), while the BACKBONE matmul (direct path to the output)
stays fp16.  Gate operands are pre-scaled by powers of two (x*8, W*32) to
center them in e4m3's normal range; the 1/256 dequant plus the sigmoid
half-angle 1/2 fold into the ACT scale (1/512).

Two-phase schedule, sized by the input-DMA roofline (~3.5MB at ~300GB/s):
the gate phase needs only the small fp8 streams (1.15MB) so it starts as
soon as they land; its chain (tg -> t -> decay) has no backbone
dependency.  The backbone phase (fp16, 2.3MB) streams in behind it.

    phase A, per j:  zg = DoubleRow-fp8 matmuls
                     tg    = Tanh(zg/512 + bg/2)              ACT
                     t     = (tg + 2*softplus'+1) * (-dt/2)   DVE STT
                     decay = Exp(t)       (ACT, one per j-pair, 2x wide)
                     em    = 1 - decay                        DVE TS
                     dech  = decay * h                        DVE
    phase B, per j:  zf = fp16 matmuls
                     f     = Tanh(zf + bb)                    ACT
                     o     = f * em + dech -> DMA out (fp16)  DVE x2

em/dech are precomputed on the otherwise-idle DVE during phase A so each
phase-B combine is only two DVE hops after its tanh — the post-last-matmul
tail stays short (the last j-tile further splits its combine in halves).
softplus(log_tau) is a [H] constant computed on the host, so the scalar
engine needs one ACT table set (tanh+exp) loaded once.  All 16-bit DVE ops
hit the 2x packed mode.

All input DMAs ride ONE HWDGE ring (sync) in exact consumption order —
concurrent rings round-robin per packet and starve whichever stream is
needed first — with small fp8 panels first so the first matmul's data
(768KB) lands as early as possible.  Tiny constants ride the gpsimd SWDGE
ring.  Output ships fp16 and is upcast on the host.
"""

from contextlib import ExitStack

import ml_dtypes
import numpy as np

import concourse.mybir as mybir
import concourse.tile as tile
from concourse import bacc
from concourse.bass_utils import run_bass_kernel_spmd

B, IN, H = 8192, 256, 512
NCORES = 8
BS = B // NCORES          # 1024 batch rows per core
KIN = IN + H              # 768 contraction dim
KT = KIN // 128           # 6 k-tiles
NP = KT // 2              # 3 k-pair panels (DoubleRow processes 2 k-tiles)
NJ = H // 128             # 4 partition tiles per output matrix
NCHUNK = 512              # matmul moving free dim per PSUM bank
NCH = BS // NCHUNK        # 2 b-chunks per core

SX = 8.0                  # gate activation pre-scale (power of 2)
SW = 32.0                 # gate weight pre-scale (power of 2)
GDEQ = 1.0 / (SX * SW * 2.0)   # ACT scale: dequant + sigmoid half-angle

F32 = mybir.dt.float32
FP16 = mybir.dt.float16
FP8 = mybir.dt.float8e4
AF = mybir.ActivationFunctionType
OP = mybir.AluOpType
PM = mybir.MatmulPerfMode

TRACE = False             # test.py flips this for profiled runs
LAST_RESULT = None        # BassKernelResults of the most recent run

_NC_CACHE = None


def _body(tc, gc8, xgc1, bc16, xhc1, ndt2, consts, outP):
    nc = tc.nc
    with ExitStack() as ctx:
        singles = ctx.enter_context(tc.tile_pool(name="singles", bufs=1))
        decs = ctx.enter_context(tc.tile_pool(name="decs", bufs=2))
        work = ctx.enter_context(tc.tile_pool(name="work", bufs=2))
        psg = ctx.enter_context(tc.tile_pool(name="psg", bufs=2, space="PSUM"))
        psf = ctx.enter_context(tc.tile_pool(name="psf", bufs=2, space="PSUM"))

        # Persistent SBUF tensors.  Activation streams are chunk-panel
        # tiles so each DMA is one contiguous-row panel in matmul
        # consumption order.
        wg_sb = singles.tile([128, KT, H], FP8, tag="wg")
        xg0_sb = singles.tile([128, KT, NCHUNK], FP8, tag="xg0")
        xg1_sb = singles.tile([128, KT, NCHUNK], FP8, tag="xg1")
        wb_sb = singles.tile([128, KT, H], FP16, tag="wb")
        xh0_sb = singles.tile([128, KT, NCHUNK], FP16, tag="xh0")
        xh1_sb = singles.tile([128, KT, NCHUNK], FP16, tag="xh1")
        ndt_sb = singles.tile([128, BS], FP16, tag="ndt")
        # cst is padded to 512B rows (the min line-rate descriptor) so it can
        # ride the main ring without tiny-descriptor poison.
        cstp = singles.tile([128, 128], F32, tag="cst")
        cst = cstp[:, 0:3 * NJ].rearrange("p (c j) -> p c j", c=3)

        wg_v = wg_sb
        xg_v = {0: xg0_sb, 1: xg1_sb}           # per-chunk moving panels
        wb_v = wb_sb
        xh_v = {0: xh0_sb, 1: xh1_sb}

        CC = KT * NCHUNK
        # All input DMAs ride ONE HWDGE ring (sync) in exact consumption
        # order: the SDMA engines round-robin between rings at packet
        # granularity, so two concurrent rings starve whichever stream is
        # needed first.  One ring drains FIFO at full fabric rate.
        nc.sync.dma_start(out=cstp, in_=consts)
        nc.sync.dma_start(out=wg_sb.rearrange("p k n -> p (k n)"),
                          in_=gc8[:, 0:CC])
        nc.sync.dma_start(out=xg0_sb.rearrange("p k c -> p (k c)"),
                          in_=gc8[:, CC:2 * CC])
        nc.sync.dma_start(out=xg1_sb.rearrange("p k c -> p (k c)"),
                          in_=xgc1)
        nc.sync.dma_start(out=ndt_sb, in_=ndt2)
        nc.sync.dma_start(out=wb_sb.rearrange("p k n -> p (k n)"),
                          in_=bc16[:, 0:CC])
        nc.sync.dma_start(out=xh0_sb.rearrange("p k c -> p (k c)"),
                          in_=bc16[:, CC:2 * CC])
        nc.sync.dma_start(out=xh1_sb.rearrange("p k c -> p (k c)"),
                          in_=xhc1)

        # --- Phase A: gate matmuls + gate chain (tg -> t -> decay) ---
        # t for j-pairs lands in one [128, 2*BS] tile so Exp runs as two
        # double-width ACTs (halves the per-instruction ACT overhead).
        # The otherwise-idle DVE then precomputes em = 1-decay and
        # dech = decay*h, so each phase-B combine is only two hops after
        # its tanh: o = f*em + dech.
        em = {}
        dech = {}
        tp = {}
        for j in range(NJ):
            zg = psg.tile([128, BS], F32, tag="zg", name=f"zg_{j}")
            for n in range(NCH):
                bsl = slice(n * NCHUNK, (n + 1) * NCHUNK)
                for g in range(NP):
                    nc.tensor.matmul(
                        zg[:, bsl],
                        wg_v[:, 2 * g:2 * g + 2, j * 128:(j + 1) * 128],
                        xg_v[n][:, 2 * g:2 * g + 2, :],
                        start=(g == 0),
                        stop=(g == NP - 1),
                        perf_mode=PM.DoubleRow,
                    )
            tg = work.tile([128, BS], FP16, tag="tg", name=f"tg_{j}")
            if j % 2 == 0:
                tp[j // 2] = work.tile([128, 2 * BS], FP16, tag="t",
                                       name=f"t_{j // 2}")
            # tg = tanh((zg + 256*bg)/512) = tanh(zg_true/2 + bg/2)
            nc.scalar.activation(
                out=tg, in_=zg, func=AF.Tanh, bias=cst[:, 0, j:j + 1],
                scale=GDEQ,
            )
            # t = (tg + (2*softplus+1)) * (-dt/2)  [= -dt * (softplus + g)]
            nc.vector.scalar_tensor_tensor(
                out=tp[j // 2][:, (j % 2) * BS:(j % 2 + 1) * BS],
                in0=tg, scalar=cst[:, 2, j:j + 1], in1=ndt_sb,
                op0=OP.add, op1=OP.mult,
            )
            if j % 2 == 1:
                dp = decs.tile([128, 2 * BS], FP16, tag="dec",
                               name=f"dec_{j // 2}")
                nc.scalar.activation(out=dp, in_=tp[j // 2], func=AF.Exp)
                ep = decs.tile([128, 2 * BS], FP16, tag="em",
                               name=f"em_{j // 2}")
                nc.vector.tensor_scalar(
                    out=ep, in0=dp, scalar1=-1.0, scalar2=1.0,
                    op0=OP.mult, op1=OP.add,
                )
                for jj in (j - 1, j):
                    em[jj] = ep[:, (jj % 2) * BS:(jj % 2 + 1) * BS]
                    dh = decs.tile([128, BS], FP16, tag="dech",
                                   name=f"dech_{jj}")
                    for c in range(NCH):
                        nc.vector.tensor_mul(
                            out=dh[:, c * NCHUNK:(c + 1) * NCHUNK],
                            in0=dp[:, (jj % 2) * BS + c * NCHUNK:
                                   (jj % 2) * BS + (c + 1) * NCHUNK],
                            in1=xh_v[c][:, 2 + jj, :],
                        )
                    dech[jj] = dh

        # --- Phase B: backbone matmuls + f + combine ---
        # Full-width combines for j<3; the last j-tile splits its epilogue
        # into one 512 chunk and two 256 half-chunks so every hop on the
        # final critical path gets cheaper.
        def combine(j, zf, csl, name):
            w = csl.stop - csl.start
            f = work.tile([128, w], FP16, tag=f"f{w}", name=f"f_{name}")
            p = work.tile([128, w], FP16, tag=f"p{w}", name=f"p_{name}")
            o = work.tile([128, w], FP16, tag=f"o{w}", name=f"o_{name}")
            nc.scalar.activation(
                out=f, in_=zf[:, csl], func=AF.Tanh, bias=cst[:, 1, j:j + 1]
            )
            # o = f*(1-decay) + decay*h, both factors precomputed in phase A
            nc.vector.tensor_mul(out=p, in0=f, in1=em[j][:, csl])
            nc.vector.tensor_add(out=o, in0=p, in1=dech[j][:, csl])
            nc.sync.dma_start(
                out=outP[:, j * BS + csl.start:j * BS + csl.stop], in_=o
            )

        for j in range(NJ):
            zf = psf.tile([128, BS], F32, tag="zf", name=f"zf_{j}")
            for n in range(NCH):
                bsl = slice(n * NCHUNK, (n + 1) * NCHUNK)
                for k in range(KT):
                    nc.tensor.matmul(
                        zf[:, bsl],
                        wb_v[:, k, j * 128:(j + 1) * 128],
                        xh_v[n][:, k, :],
                        start=(k == 0),
                        stop=(k == KT - 1),
                    )
            if j < NJ - 1:
                combine(j, zf, slice(0, BS), f"{j}")
            else:
                # Last tile: one full-width tanh, then quarter-width V
                # chains + DMAs so the final hops are as cheap as possible.
                f = work.tile([128, BS], FP16, tag="f1024", name="f_3")
                nc.scalar.activation(
                    out=f, in_=zf, func=AF.Tanh, bias=cst[:, 1, j:j + 1]
                )
                for q in range(2):
                    csl = slice(q * (BS // 2), (q + 1) * (BS // 2))
                    w = BS // 2
                    p = work.tile([128, w], FP16, tag="pq", name=f"p_3_{q}")
                    o = work.tile([128, w], FP16, tag="oq", name=f"o_3_{q}")
                    nc.vector.tensor_mul(out=p, in0=f[:, csl],
                                         in1=em[j][:, csl])
                    nc.vector.tensor_add(out=o, in0=p, in1=dech[j][:, csl])
                    nc.sync.dma_start(
                        out=outP[:, j * BS + csl.start:j * BS + csl.stop],
                        in_=o
                    )


def build_nc():
    nc = bacc.Bacc(
        "TRN2",
        target_bir_lowering=False,
        debug=False,
        enable_asserts=False,
        num_devices=NCORES,
    )
    # Partition-major packed streams: row p holds that partition's entire
    # contiguous payload.
    CC = KT * NCHUNK
    gc8 = nc.dram_tensor("gc8", [128, 2 * CC], FP8, kind="ExternalInput").ap()
    xgc1 = nc.dram_tensor("xgc1", [128, CC], FP8, kind="ExternalInput").ap()
    bc16 = nc.dram_tensor("bc16", [128, 2 * CC], FP16, kind="ExternalInput").ap()
    xhc1 = nc.dram_tensor("xhc1", [128, CC], FP16, kind="ExternalInput").ap()
    ndt2 = nc.dram_tensor("ndt2", [128, BS], FP16, kind="ExternalInput").ap()
    consts = nc.dram_tensor("consts", [128, 128], F32,
                            kind="ExternalInput").ap()
    outP = nc.dram_tensor("outP", [128, NJ * BS], FP16, kind="ExternalOutput").ap()
    with tile.TileContext(nc) as tc:
        _body(tc, gc8, xgc1, bc16, xhc1, ndt2, consts, outP)
    nc.compile()
    return nc


def _get_nc():
    global _NC_CACHE
    if _NC_CACHE is None:
        _NC_CACHE = build_nc()
    return _NC_CACHE


def _pack_cmajor(a, kt, nch, nchunk):
    """[kt*128, nch*nchunk] -> [128, nch*kt*nchunk] chunk-major pack: row p
    holds [chunk0: k0..k5 | chunk1: k0..k5], each 128-partition-sliced."""
    return np.ascontiguousarray(
        a.reshape(kt, 128, nch, nchunk).transpose(1, 2, 0, 3)
        .reshape(128, nch * kt * nchunk)
    )


def _pack_pmajor(a, kt):
    """[kt*128, C] -> [128, kt*C]: partition-major pack so each of the 128
    DMA rows is contiguous in DRAM."""
    c = a.shape[1]
    return np.ascontiguousarray(
        a.reshape(kt, 128, c).transpose(1, 0, 2).reshape(128, kt * c)
    )


def make_in_maps(x, h, delta_t, W_backbone, b_backbone, W_gx, b_gx, W_gh,
                 gate_b, log_tau):
    f32 = np.float32
    xh = np.concatenate(
        [np.asarray(x, f32), np.asarray(h, f32)], axis=1
    )                                                   # [B, 768]
    xhT = np.ascontiguousarray(xh.T)                    # [768, B] f32
    xh16 = xhT.astype(np.float16)
    xg8 = np.asarray(xhT * SX, dtype=ml_dtypes.float8_e4m3)

    WgT = np.concatenate(
        [np.asarray(W_gx, f32), np.asarray(W_gh, f32)], axis=1
    ).T                                                 # [768, H]
    w8g = _pack_pmajor(np.asarray(WgT * SW, dtype=ml_dtypes.float8_e4m3), KT)
    w16b = _pack_pmajor(
        np.ascontiguousarray(np.asarray(W_backbone, f32).T).astype(np.float16),
        KT,
    )

    sp2 = 2.0 * np.log1p(np.exp(np.asarray(log_tau, f32))) + 1.0
    # cstP[p, c*NJ+j] = const_c[j*128+p]
    cstv = np.stack(
        [
            (np.asarray(b_gx, f32) + np.asarray(gate_b, f32)) * 0.5,
            np.asarray(b_backbone, f32),
            sp2,
        ]
    )                                                   # [3, H]
    cstP = np.zeros((128, 128), f32)
    cstP[:, 0:3 * NJ] = (
        cstv.reshape(3, NJ, 128).transpose(2, 0, 1).reshape(128, 3 * NJ)
    )
    ndt2 = (np.asarray(delta_t, f32) * -0.5).astype(np.float16)   # [B]

    CC = KT * NCHUNK
    in_maps = []
    for c in range(NCORES):
        sl = slice(c * BS, (c + 1) * BS)
        xgp = _pack_cmajor(xg8[:, sl], KT, NCH, NCHUNK)
        xhp = _pack_cmajor(xh16[:, sl], KT, NCH, NCHUNK)
        in_maps.append(
            {
                "gc8": np.concatenate([w8g, xgp[:, 0:CC]], axis=1),
                "xgc1": np.ascontiguousarray(xgp[:, CC:2 * CC]),
                "bc16": np.concatenate([w16b, xhp[:, 0:CC]], axis=1),
                "xhc1": np.ascontiguousarray(xhp[:, CC:2 * CC]),
                "ndt2": np.ascontiguousarray(
                    np.broadcast_to(ndt2[sl][None, :], (128, BS))
                ),
                "consts": cstP,
            }
        )
    return in_maps


def kernel(x, h, delta_t, W_backbone, b_backbone, W_gx, b_gx, W_gh, gate_b,
           log_tau):
    global LAST_RESULT
    in_maps = make_in_maps(x, h, delta_t, W_backbone, b_backbone, W_gx, b_gx,
                           W_gh, gate_b, log_tau)
    nc = _get_nc()
    res = run_bass_kernel_spmd(
        nc, in_maps, core_ids=list(range(NCORES)), trace=TRACE
    )
    LAST_RESULT = res
    # outP is [128, NJ*BS] partition-major; unpack to [H, BS] then gather.
    outs = []
    for r in res.results:
        op = r["outP"].reshape(128, NJ, BS).transpose(1, 0, 2).reshape(H, BS)
        outs.append(op)
    out = np.concatenate(outs, axis=1).T
    return np.ascontiguousarray(out).astype(np.float32)
